# revision 1
# baseline (speedup 1.0000x reference)
"""Trainium2 Bass kernel for nn_Correction_Module_dense.

Computation (bit-exact with the jax reference):
    grad   = x - roll(x, 1, axis=1)              # circular diff along neuron axis
    lower  = mean_grad - k*sqrt(var_grad)        # per-neuron, computed on host
    upper  = mean_grad + k*sqrt(var_grad)
    y      = x * (grad >= lower) * (grad <= upper)

Sharding: pure data parallel over the batch dim; 8 cores x [512, 8192] slabs.
Layout: batch rows -> partitions, neurons -> free axis (circular diff is a
free-dim offset AP).  lower/upper are broadcast once into [128, n] SBUF
tensors by log2-doubling SBUF->SBUF DMAs.

Raw-bass implementation (explicit semaphores): the toolchain's walrus codegen
allows only one inline sync-wait per compute instruction, which breaks
TileContext's packed waits for this dependency pattern; raw blocks emit
stand-alone wait_ge instructions instead.

Engine split per column-chunk:
    Pool (gpsimd): g = x - x_shift
    DVE (vector):  p = g >= lower; q = g <= upper; r = p*q (in place); y = r*x
    SP (sync):     all DMAs (loads, broadcast, stores)
"""

import numpy as np

import concourse.bass as bass
import concourse.mybir as mybir

B, N = 4096, 8192
N_CORES = 8
ROWS = B // N_CORES  # rows per core
P = 128


def build_nc(rows=ROWS, n=N, chunk=1024):
    nt = rows // P          # row tiles
    nch = n // chunk        # chunks per row tile
    f32 = mybir.dt.float32
    sub = mybir.AluOpType.subtract
    mul = mybir.AluOpType.mult
    is_ge = mybir.AluOpType.is_ge
    is_le = mybir.AluOpType.is_le

    XB = 2   # xt buffers
    YB = 4   # ym buffers
    GB = 2   # g buffers

    nc = bass.Bass()
    x = nc.dram_tensor("x", [rows, n], f32, kind="ExternalInput")
    low = nc.dram_tensor("low", [n], f32, kind="ExternalInput")
    up = nc.dram_tensor("up", [n], f32, kind="ExternalInput")
    y = nc.dram_tensor("y", [rows, n], f32, kind="ExternalOutput")

    from contextlib import ExitStack

    with ExitStack() as ctx:
        blow = ctx.enter_context(nc.sbuf_tensor("blow", [P, n], f32))
        bup = ctx.enter_context(nc.sbuf_tensor("bup", [P, n], f32))
        xt = [
            ctx.enter_context(nc.sbuf_tensor(f"xt{i}", [P, n], f32))
            for i in range(XB)
        ]
        g = [
            ctx.enter_context(nc.sbuf_tensor(f"g{i}", [P, chunk], f32))
            for i in range(GB)
        ]
        pm = [
            ctx.enter_context(nc.sbuf_tensor(f"pm{i}", [P, chunk], f32))
            for i in range(GB)
        ]
        qm = [
            ctx.enter_context(nc.sbuf_tensor(f"qm{i}", [P, chunk], f32))
            for i in range(GB)
        ]
        rm = [
            ctx.enter_context(nc.sbuf_tensor(f"rm{i}", [P, chunk], f32))
            for i in range(GB)
        ]
        ym = [
            ctx.enter_context(nc.sbuf_tensor(f"ym{i}", [P, chunk], f32))
            for i in range(YB)
        ]
        # One in-flight DMA per semaphore so sem-threshold waits are safe
        # under out-of-order DMA completion.
        LB = ctx.enter_context(nc.semaphore("LB"))  # broadcast chain (x16)
        Lb = [ctx.enter_context(nc.semaphore(f"Lb{i}")) for i in range(XB)]
        Sb = [ctx.enter_context(nc.semaphore(f"Sb{i}")) for i in range(YB)]
        PS = ctx.enter_context(nc.semaphore("PS"))  # pool g-chunk progress
        V = ctx.enter_context(nc.semaphore("V"))  # dve y-chunk progress
        block = ctx.enter_context(nc.Block())

        # 8 broadcast DMAs per bounds tensor: 1 load + 7 doublings
        n_bcast = 2 * 8
        l_bcast = 16 * n_bcast
        assert nch % YB == 0
        spt = nch // YB  # stores per ym buffer per row tile

        @block.sync
        def _(sync):
            lv = 0
            for vec, t in ((low, blow), (up, bup)):
                sync.dma_start(out=t[0:1, :], in_=vec[None, :]).then_inc(LB, 16)
                lv += 16
                pcnt = 1
                while pcnt < P:
                    sync.wait_ge(LB, lv)
                    sync.dma_start(
                        out=t[pcnt : 2 * pcnt, :], in_=t[0:pcnt, :]
                    ).then_inc(LB, 16)
                    lv += 16
                    pcnt *= 2
            for t in range(nt):
                if t >= XB:
                    # xt[t % XB] reusable once tile t-XB fully stored
                    for i in range(YB):
                        sync.wait_ge(Sb[i], 16 * spt * (t - XB + 1))
                sync.dma_start(
                    out=xt[t % XB][:], in_=x[t * P : (t + 1) * P, :]
                ).then_inc(Lb[t % XB], 16)
                for c in range(nch):
                    idx = t * nch + c
                    sync.wait_ge(V, idx + 1)
                    sync.dma_start(
                        out=y[t * P : (t + 1) * P, c * chunk : (c + 1) * chunk],
                        in_=ym[idx % YB][:],
                    ).then_inc(Sb[idx % YB], 16)

        @block.gpsimd
        def _(gpsimd):
            for t in range(nt):
                gpsimd.wait_ge(Lb[t % XB], 16 * (t // XB + 1))
                xb = xt[t % XB]
                for c in range(nch):
                    idx = t * nch + c
                    if idx >= GB:
                        gpsimd.wait_ge(V, idx - GB + 1)
                    gb = g[idx % GB]
                    c0 = c * chunk
                    if c == 0:
                        gpsimd.tensor_tensor(
                            gb[:, 1:chunk], xb[:, 1:chunk], xb[:, 0 : chunk - 1], sub
                        )
                        gpsimd.tensor_tensor(
                            gb[:, 0:1], xb[:, 0:1], xb[:, n - 1 : n], sub
                        ).then_inc(PS, 1)
                    else:
                        gpsimd.tensor_tensor(
                            gb[:], xb[:, c0 : c0 + chunk], xb[:, c0 - 1 : c0 + chunk - 1], sub
                        ).then_inc(PS, 1)

        @block.vector
        def _(vector):
            vector.wait_ge(LB, l_bcast)
            for t in range(nt):
                vector.wait_ge(Lb[t % XB], 16 * (t // XB + 1))
                xb = xt[t % XB]
                for c in range(nch):
                    idx = t * nch + c
                    c0 = c * chunk
                    gb = g[idx % GB]
                    pb = pm[idx % GB]
                    qb = qm[idx % GB]
                    rb = rm[idx % GB]
                    yb = ym[idx % YB]
                    vector.wait_ge(PS, idx + 1)
                    if idx >= YB:
                        vector.wait_ge(Sb[idx % YB], 16 * (idx // YB))
                    vector.tensor_tensor(pb[:], gb[:], blow[:, c0 : c0 + chunk], is_ge)
                    vector.tensor_tensor(qb[:], gb[:], bup[:, c0 : c0 + chunk], is_le)
                    vector.drain()
                    vector.tensor_tensor(rb[:], pb[:], qb[:], mul)
                    vector.drain()
                    vector.tensor_tensor(
                        yb[:], rb[:], xb[:, c0 : c0 + chunk], mul
                    ).then_inc(V, 1)

    return nc


def _host_bounds(mean_grad, var_grad, k):
    mg = np.asarray(mean_grad, dtype=np.float32)
    vg = np.asarray(var_grad, dtype=np.float32)
    kf = np.float32(k)
    std = np.sqrt(vg, dtype=np.float32)
    ks = (kf * std).astype(np.float32)
    lower = (mg - ks).astype(np.float32)
    upper = (mg + ks).astype(np.float32)
    return lower, upper


_NC_CACHE = {}


def kernel(output, mean_grad, var_grad, k):
    from concourse.bass_utils import run_bass_kernel_spmd

    x = np.ascontiguousarray(np.asarray(output, dtype=np.float32))
    assert x.shape == (B, N), x.shape
    lower, upper = _host_bounds(mean_grad, var_grad, k)

    if "nc" not in _NC_CACHE:
        _NC_CACHE["nc"] = build_nc()
    nc = _NC_CACHE["nc"]

    in_maps = [
        {"x": x[i * ROWS : (i + 1) * ROWS], "low": lower, "up": upper}
        for i in range(N_CORES)
    ]
    res = run_bass_kernel_spmd(nc, in_maps, core_ids=list(range(N_CORES)))
    return np.concatenate([res.results[i]["y"] for i in range(N_CORES)], axis=0)



# revision 2
# speedup vs baseline: 1.2053x; 1.2053x over previous
"""Trainium2 Bass kernel for nn_Correction_Module_dense.

Math (equivalent to the jax reference):
    g    = x - roll(x, 1, axis=1)            # circular diff along neuron axis
    mask = |g - mean_grad| <= k*sqrt(var_grad)
    y    = x * mask

Sharding: pure data parallel over batch; 8 cores x [512, 8192] slabs.

Per-core pipeline, [128, 1024] chunks (32 chunk-steps).  GPSIMD's walrus
codegen only accepts add/subtract/mult tensor_tensor, so:
    SP   : all DMAs (quarter-granularity x loads; tile-0's first quarter is
           split so compute starts ~5 us in).  xt column 0 holds x[:, N-1]
           (wrap), making g a uniform shifted-AP subtract.
    PE   : per-neuron bound broadcast via K=3 bf16 matmuls
           ones[3,128]^T @ split[3,512] -> PSUM; the rows are a hi/mid/lo
           bf16 split of the f32 vector, reconstructed exactly by the f32
           PSUM accumulation.  No DMA traffic.
    ACT  : PSUM->SBUF broadcast copies + a = |d| (Abs) in place.
    Pool : g = x - xshift (all chunks) + d = g + (-mean_b) for POOL_D chunks.
    DVE  : d for the rest, m = (|d| <= ks_b), y = m * x.

d-completion uses two sems (DD: DVE, DP: Pool) so each stays monotonic in
chunk order.  Engine busy (cost model): DMA 93.7us, DVE ~94.7, Pool ~93.6,
ACT 50, PE 13.7 -- right at the 16+16 MiB HBM roofline.  Same-engine dep
pairs rely on in-order engine execution (HW auto-drains between ops);
drains=True adds explicit drains for CoreSim's conservative race detector.
"""

import numpy as np

import concourse.bass as bass
import concourse.mybir as mybir

B, N = 4096, 8192
N_CORES = 8
ROWS = B // N_CORES   # 512 rows per core
P = 128
NT = ROWS // P        # 4 row tiles
CHUNK = 1024
NCH = N // CHUNK      # 8 chunks per row tile
NIDX = NT * NCH       # 32 chunk-steps per core
R = 512               # PSUM broadcast range (one bank)
Q = 2048              # load-quarter width
POOL_D = (2, 5, 7)    # chunks whose d runs on Pool (rest on DVE)

f32 = mybir.dt.float32
bf16 = mybir.dt.bfloat16


def build_nc(pool_d=POOL_D, drains=True):
    sub = mybir.AluOpType.subtract
    add = mybir.AluOpType.add
    mult = mybir.AluOpType.mult
    is_le = mybir.AluOpType.is_le
    Abs = mybir.ActivationFunctionType.Abs
    Copy = mybir.ActivationFunctionType.Copy

    nc = bass.Bass(detect_race_conditions=drains)
    x = nc.dram_tensor("x", [ROWS, N], f32, kind="ExternalInput")
    # vecd: [3, 2N+128] bf16 = hi/mid/lo splits of -mean | k*sqrt(var) | ones
    vecd = nc.dram_tensor("vecd", [3, 2 * N + P], bf16, kind="ExternalInput")
    y = nc.dram_tensor("y", [ROWS, N], f32, kind="ExternalOutput")

    pd_set = {i for i in range(NIDX) if i % NCH in pool_d}

    def ndd(idx):
        """DVE-computed d's with id <= idx."""
        return sum(1 for i in range(idx + 1) if i not in pd_set)

    def ndp(idx):
        return sum(1 for i in range(idx + 1) if i in pd_set)

    from contextlib import ExitStack

    with ExitStack() as ctx:
        sb = lambda name, shape, dt=f32: ctx.enter_context(
            nc.sbuf_tensor(name, shape, dt)
        )
        xt = [sb(f"xt{i}", [P, N + 1]) for i in range(2)]
        mean_b = sb("mean_b", [P, N])   # holds -mean (host negates)
        ks_b = sb("ks_b", [P, N])
        vec = sb("vec", [3, 2 * N + P], bf16)
        msp = vec[:, 0:N]
        ksp = vec[:, N : 2 * N]
        ones = vec[:, 2 * N : 2 * N + P]
        gb = [sb(f"g{i}", [P, CHUNK]) for i in range(3)]
        db = [sb(f"d{i}", [P, CHUNK]) for i in range(2)]   # d then |d| in place
        mb = [sb(f"m{i}", [P, CHUNK]) for i in range(2)]
        ym = [sb(f"ym{i}", [P, CHUNK]) for i in range(4)]
        ps = [ctx.enter_context(nc.psum_tensor(f"ps{i}", [P, 2 * R], f32))
              for i in range(2)]

        sem = lambda name: ctx.enter_context(nc.semaphore(name))
        LV = sem("LV")       # vec load (1 DMA x16)
        E0 = sem("E0")       # tile-0 wrap + first eighth (2 DMAs x16)
        LQ = [[sem(f"LQ{s}_{q}") for q in range(4)] for s in range(2)]
        BB = sem("BB")       # PE matmul done (per matmul)
        C = sem("C")         # ACT bcast pair copy done (per 1024-range pair)
        PG = sem("PG")       # Pool g done (per chunk)
        DD = sem("DD")       # DVE d done (count of DVE-d's)
        DP = sem("DP")       # Pool d done (count of Pool-d's)
        A = sem("A")         # ACT |d| done (per chunk)
        Mm = sem("Mm")       # DVE m done (per chunk)
        V = sem("V")         # DVE y done (per chunk)
        S = [sem(f"S{i}") for i in range(4)]   # stores (x16)

        block = ctx.enter_context(nc.Block())

        # ---- load planning -------------------------------------------------
        def tile_plan(t):
            s = t % 2
            if t == 0:
                return [
                    ("wrap", E0), (0, CHUNK, E0),
                    (CHUNK, Q, LQ[s][0]),
                    (Q, 2 * Q, LQ[s][1]),
                    (2 * Q, 3 * Q, LQ[s][2]),
                    (3 * Q, 4 * Q, LQ[s][3]),
                ]
            return [
                ("wrap", LQ[s][0]), (0, Q, LQ[s][0]),
                (Q, 2 * Q, LQ[s][1]),
                (2 * Q, 3 * Q, LQ[s][2]),
                (3 * Q, 4 * Q, LQ[s][3]),
            ]

        plans = {t: tile_plan(t) for t in range(NT)}

        # g(t, c) reads x columns [c*CHUNK-1, (c+1)*CHUNK) (wrap for c == 0)
        sem_count = {}
        g_waits = {}
        for t in range(NT):
            seg_done = []
            for seg in plans[t]:
                semh = seg[-1]
                sem_count[id(semh)] = sem_count.get(id(semh), 0) + 16
                cs, ce = (-1, 0) if seg[0] == "wrap" else (seg[0], seg[1])
                seg_done.append((cs, ce, semh, sem_count[id(semh)]))
            for c in range(NCH):
                lo = c * CHUNK - 1
                hi = (c + 1) * CHUNK
                waits = {}
                for cs, ce, semh, cnt in seg_done:
                    if cs < hi and ce > lo:
                        key = id(semh)
                        if key not in waits or waits[key][1] < cnt:
                            waits[key] = (semh, cnt)
                g_waits[(t, c)] = list(waits.values())

        @block.sync
        def _(sync):
            def emit_loads(t, segs):
                rows = x[t * P : (t + 1) * P]
                s = t % 2
                for seg in segs:
                    semh = seg[-1]
                    if seg[0] == "wrap":
                        with nc.allow_non_contiguous_dma(reason="wrap col"):
                            sync.dma_start(
                                out=xt[s][:, 0:1], in_=rows[:, N - 1 : N]
                            ).then_inc(semh, 16)
                    else:
                        cs, ce = seg[0], seg[1]
                        sync.dma_start(
                            out=xt[s][:, 1 + cs : 1 + ce], in_=rows[:, cs:ce]
                        ).then_inc(semh, 16)

            emit_loads(0, plans[0][:2])       # wrap + first eighth
            sync.dma_start(out=vec[:], in_=vecd[:]).then_inc(LV, 16)
            emit_loads(0, plans[0][2:])
            emit_loads(1, plans[1])
            for idx in range(NIDX):
                t, c = divmod(idx, NCH)
                sync.wait_ge(V, idx + 1)
                sync.dma_start(
                    out=y[t * P : (t + 1) * P, c * CHUNK : (c + 1) * CHUNK],
                    in_=ym[idx % 4][:],
                ).then_inc(S[idx % 4], 16)
                # tile t+2 loads stream in as slot quarters free up:
                # store (t, 2q+2)'s V-wait implies y(t, 2q+2) done.
                if t + 2 < NT and c in (2, 4, 6, 7):
                    qi = {2: 0, 4: 1, 6: 2, 7: 3}[c]
                    segs = plans[t + 2]
                    if qi == 0:
                        emit_loads(t + 2, segs[:2])
                    else:
                        emit_loads(t + 2, segs[qi + 1 : qi + 2])

        @block.tensor
        def _(tensor):
            # broadcast pairs: p = 2*rr + (0: -mean, 1: ks), rr a 1024-range
            tensor.wait_ge(LV, 16)
            for p in range(2 * NCH):
                rr, which = divmod(p, 2)
                src = msp if which == 0 else ksp
                if p >= 2:
                    tensor.wait_ge(C, p - 1)  # ACT copied ps[p%2], reusable
                for h in range(2):
                    r0 = rr * CHUNK + h * R
                    tensor.matmul(
                        ps[p % 2][:, h * R : (h + 1) * R],
                        ones,
                        src[:, r0 : r0 + R],
                        start=True,
                        stop=True,
                    ).then_inc(BB, 1)

        @block.scalar
        def _(scalar):
            q = 0

            def copies(k):
                nonlocal q
                for _ in range(k):
                    rr, which = divmod(q, 2)
                    dst = mean_b if which == 0 else ks_b
                    scalar.wait_ge(BB, 2 * q + 2)  # both halves of pair q
                    scalar.activation(
                        dst[:, rr * CHUNK : (rr + 1) * CHUNK], ps[q % 2][:], Copy
                    ).then_inc(C, 1)
                    q += 1

            for idx in range(NIDX):
                if q < 2 * NCH:
                    copies(2)
                if idx in pd_set:
                    scalar.wait_ge(DP, ndp(idx))
                else:
                    scalar.wait_ge(DD, ndd(idx))
                # |d| in place: db[idx%2] both source and destination
                scalar.activation(db[idx % 2][:], db[idx % 2][:], Abs).then_inc(A, 1)

        @block.gpsimd
        def _(gpsimd):
            # step i: g(i), then d(i-1) when (i-1) is a Pool-d chunk
            for i in range(NIDX + 1):
                if i < NIDX:
                    t, c = divmod(i, NCH)
                    for semh, thresh in g_waits[(t, c)]:
                        gpsimd.wait_ge(semh, thresh)
                    if i >= 3 and (i - 3) not in pd_set:
                        gpsimd.wait_ge(DD, ndd(i - 3))  # gb[i%3] free
                    # (i-3) in pd_set: Pool's own d(i-3) precedes in order
                    if drains and i >= 3 and (i - 3) in pd_set:
                        gpsimd.drain()  # WAR: own d(i-3) read gb[i%3]
                    c0 = c * CHUNK
                    gpsimd.tensor_tensor(
                        gb[i % 3][:],
                        xt[t % 2][:, c0 + 1 : c0 + CHUNK + 1],
                        xt[t % 2][:, c0 : c0 + CHUNK],
                        sub,
                    ).then_inc(PG, 1)
                j = i - 1
                if 0 <= j < NIDX and j in pd_set:
                    tj, cj = divmod(j, NCH)
                    cj0 = cj * CHUNK
                    gpsimd.wait_ge(C, 2 * cj + 1)
                    if j >= 2:
                        gpsimd.wait_ge(Mm, j - 1)  # db[j%2] free (m(j-2) done)
                    if drains:
                        gpsimd.drain()  # RAW: reads gb[j%3] from own g(j)
                    # mean_b holds -mean, so d = g + mean_b
                    gpsimd.tensor_tensor(
                        db[j % 2][:], gb[j % 3][:],
                        mean_b[:, cj0 : cj0 + CHUNK], add,
                    ).then_inc(DP, 1)

        @block.vector
        def _(vector):
            # step i: m(i-2), d(i), y(i-3)   (m before d: db[i%2] WAR)
            for i in range(NIDX + 3):
                j = i - 2
                if 0 <= j < NIDX:
                    tj, cj = divmod(j, NCH)
                    vector.wait_ge(A, j + 1)
                    vector.wait_ge(C, 2 * cj + 2)
                    if drains:
                        vector.drain()  # mb[j%2] WAR vs y(j-2); db read
                    vector.tensor_tensor(
                        mb[j % 2][:],
                        db[j % 2][:],
                        ks_b[:, cj * CHUNK : (cj + 1) * CHUNK],
                        is_le,
                    ).then_inc(Mm, 1)
                if i < NIDX and i not in pd_set:
                    t, c = divmod(i, NCH)
                    c0 = c * CHUNK
                    vector.wait_ge(PG, i + 1)
                    vector.wait_ge(C, 2 * c + 1)
                    if i >= 3 and (i - 3) in pd_set:
                        vector.wait_ge(DP, ndp(i - 3))  # gb[i%3] free
                    # (i-3) DVE-d: own order.  db[i%2] free: m(i-2) precedes.
                    if drains:
                        vector.drain()
                    vector.tensor_tensor(
                        db[i % 2][:], gb[i % 3][:], mean_b[:, c0 : c0 + CHUNK], add
                    ).then_inc(DD, 1)
                jy = i - 3
                if 0 <= jy < NIDX:
                    ty, cy = divmod(jy, NCH)
                    cy0 = cy * CHUNK
                    vector.wait_ge(Mm, jy + 1)
                    if jy >= 4:
                        vector.wait_ge(S[jy % 4], 16 * (jy // 4))  # ym free
                    if drains:
                        vector.drain()
                    vector.tensor_tensor(
                        ym[jy % 4][:],
                        mb[jy % 2][:],
                        xt[ty % 2][:, cy0 + 1 : cy0 + CHUNK + 1],
                        mult,
                    ).then_inc(V, 1)

    return nc


def _host_vectors(mean_grad, var_grad, k):
    import ml_dtypes

    mg = np.asarray(mean_grad, dtype=np.float32)
    vg = np.asarray(var_grad, dtype=np.float32)
    kf = np.float32(k)
    ks = (kf * np.sqrt(vg, dtype=np.float32)).astype(np.float32)

    def split3(v):
        hi = v.astype(ml_dtypes.bfloat16)
        r1 = v - hi.astype(np.float32)
        mid = r1.astype(ml_dtypes.bfloat16)
        r2 = r1 - mid.astype(np.float32)
        lo = r2.astype(ml_dtypes.bfloat16)
        return np.stack([hi, mid, lo])

    vec = np.empty((3, 2 * N + P), dtype=ml_dtypes.bfloat16)
    vec[:, 0:N] = split3(-mg)
    vec[:, N : 2 * N] = split3(ks)
    vec[:, 2 * N :] = np.ones((3, P), dtype=ml_dtypes.bfloat16)
    return vec


class _FastRunner:
    """Cached PJRT dispatch (axon path).

    run_bass_kernel_spmd -> run_bass_via_pjrt rebuilds jax.jit(shard_map(...))
    every call (retrace), transfers 128 MiB of host zeros for the donated
    outputs, and splits/reconcatenates the output.  This does the lowering
    once, keeps the compiled callable, creates the donated zeros on device,
    and feeds the full [4096, 8192] input directly.
    """

    def __init__(self, nc, n_cores):
        import jax
        import jax.numpy as jnp
        from jax.sharding import Mesh, NamedSharding, PartitionSpec
        from jax.experimental.shard_map import shard_map
        from concourse import bass2jax
        import concourse.mybir as mybir

        bass2jax.install_neuronx_cc_hook()
        in_names = []
        out_names = []
        out_avals = []
        zero_shapes = []
        partition_name = (
            nc.partition_id_tensor.name if nc.partition_id_tensor else None
        )
        for alloc in nc.m.functions[0].allocations:
            if not isinstance(alloc, mybir.MemoryLocationSet):
                continue
            name = alloc.memorylocations[0].name
            if alloc.kind == "ExternalInput":
                if name != partition_name:
                    in_names.append(name)
            elif alloc.kind == "ExternalOutput":
                shape = tuple(alloc.tensor_shape)
                dtype = mybir.dt.np(alloc.dtype)
                out_names.append(name)
                out_avals.append(jax.core.ShapedArray(shape, dtype))
                zero_shapes.append((shape, dtype))
        if nc.dbg_addr is not None:
            raise RuntimeError("debug nc unsupported in fast path")
        self.in_names = in_names
        n_params = len(in_names)
        n_outs = len(out_names)
        all_in_names = list(in_names) + list(out_names)
        if partition_name is not None:
            all_in_names.append(partition_name)

        def _body(*args):
            operands = list(args)
            if partition_name is not None:
                operands.append(bass2jax.partition_id_tensor())
            outs = bass2jax._bass_exec_p.bind(
                *operands,
                out_avals=tuple(out_avals),
                in_names=tuple(all_in_names),
                out_names=tuple(out_names),
                lowering_input_output_aliases=(),
                sim_require_finite=True,
                sim_require_nnan=True,
                nc=nc,
            )
            return tuple(outs)

        devices = jax.devices()[:n_cores]
        assert len(devices) == n_cores, len(jax.devices())
        mesh = Mesh(np.asarray(devices), ("core",))
        spec = PartitionSpec("core")
        self._sharded = jax.jit(
            shard_map(
                _body,
                mesh=mesh,
                in_specs=(spec,) * (n_params + n_outs),
                out_specs=(spec,) * n_outs,
                check_rep=False,
            ),
            donate_argnums=tuple(range(n_params, n_params + n_outs)),
            keep_unused=True,
        )
        sharding = NamedSharding(mesh, spec)
        self._make_zeros = jax.jit(
            lambda: tuple(
                jnp.zeros((n_cores * s[0], *s[1:]), d) for s, d in zero_shapes
            ),
            out_shardings=(sharding,) * n_outs,
        )

    def __call__(self, *global_inputs):
        zeros = self._make_zeros()
        outs = self._sharded(*global_inputs, *zeros)
        return [np.asarray(o) for o in outs]


_CACHE = {}


def _run_fallback(nc, x, vec):
    from concourse.bass_utils import run_bass_kernel_spmd

    in_maps = [
        {"x": x[i * ROWS : (i + 1) * ROWS], "vecd": vec} for i in range(N_CORES)
    ]
    res = run_bass_kernel_spmd(nc, in_maps, core_ids=list(range(N_CORES)))
    return np.concatenate([res.results[i]["y"] for i in range(N_CORES)], axis=0)


def kernel(output, mean_grad, var_grad, k):
    x = np.ascontiguousarray(np.asarray(output, dtype=np.float32))
    assert x.shape == (B, N), x.shape
    vec = _host_vectors(mean_grad, var_grad, k)

    if "nc" not in _CACHE:
        _CACHE["nc"] = build_nc(drains=False)
    nc = _CACHE["nc"]

    try:
        if "runner" not in _CACHE:
            _CACHE["runner"] = _FastRunner(nc, N_CORES)
        runner = _CACHE["runner"]
        vec8 = np.ascontiguousarray(np.tile(vec, (N_CORES, 1)))
        ins = {"x": x, "vecd": vec8}
        outs = runner(*[ins[nm] for nm in runner.in_names])
        return outs[0]
    except Exception:
        _CACHE.pop("runner", None)
        return _run_fallback(nc, x, vec)
